# revision 1
# baseline (speedup 1.0000x reference)
"""Trainium kernel for nn_PPOCoLightNetwork.

Strategy: data-parallel over the leading batch dim (B=8) across the 8
NeuronCores — one batch element per core, params replicated. Each core
runs the full per-sample network (embed MLP -> GAT with one-hot neighbor
gather + edge-distance bias -> phase-competition actor head -> critic).
Outputs are gathered back to full shape on host.
"""
import numpy as np
import jax
import jax.numpy as jnp

A, D_IN = 1024, 32
HID = 128
H_DIM, HEADS = 32, 5
P, PD, PH = 8, 64, 4
ROW = COL = 32
N = 5


def _ln(x, g, b, eps=1e-5):
    m = jnp.mean(x, axis=-1, keepdims=True)
    v = jnp.mean(jnp.square(x - m), axis=-1, keepdims=True)
    return (x - m) / jnp.sqrt(v + eps) * g + b


def _per_core(obs, adj_matrix, phase_axis, p):
    # obs [A,D_IN], adj [A,N,A], phase [A]
    h = obs
    for i in range(2):
        h = _ln(jax.nn.relu(h @ p[f'emb_w{i}'] + p[f'emb_b{i}']),
                p[f'emb_g{i}'], p[f'emb_be{i}'])
    # GAT: one-hot neighbor gather as dense matmul
    nbr = jnp.einsum('anm,md->and', adj_matrix, h)                 # [A,N,HID]
    q = (h @ p['gq_w'] + p['gq_b']).reshape(A, H_DIM, HEADS).transpose(0, 2, 1)
    k = (nbr @ p['gk_w'] + p['gk_b']).reshape(A, N, H_DIM, HEADS).transpose(0, 3, 1, 2)
    v = (nbr @ p['gv_w'] + p['gv_b']).reshape(A, N, H_DIM, HEADS).transpose(0, 3, 1, 2)
    logits = jnp.einsum('ahd,ahnd->ahn', q, k)                     # [A,H,N]
    # edge-distance bias
    idx = jnp.arange(A)
    coords = jnp.stack([(idx // COL).astype(jnp.float32),
                        (idx % COL).astype(jnp.float32)], axis=-1)  # [A,2]
    nidx = jnp.argmax(adj_matrix, axis=-1)                          # [A,N]
    nrc = coords[nidx]                                              # [A,N,2]
    erc = coords[:, None, :]                                        # [A,1,2]
    dr = nrc[..., 0] - erc[..., 0]
    dc = nrc[..., 1] - erc[..., 1]
    man = jnp.abs(dr) + jnp.abs(dc)
    dist = man / float(ROW + COL - 2)
    nh = (jnp.abs(dc) >= jnp.abs(dr)).astype(jnp.float32)
    ph = jnp.broadcast_to(phase_axis[:, None], nh.shape)
    same = (nh == ph).astype(jnp.float32)
    conn = (man > 0).astype(jnp.float32)
    eraw = jnp.stack([same, 1.0 - same, dist, conn], axis=-1)       # [A,N,4]
    ebias = jax.nn.relu(eraw @ p['ew1'] + p['eb1']) @ p['ew2'] + p['eb2']
    logits = logits + ebias.transpose(0, 2, 1)
    att = jax.nn.softmax(logits, axis=-1)
    agg = jnp.einsum('ahn,ahnd->ahd', att, v).mean(axis=1)          # [A,H_DIM]
    h2 = jax.nn.relu(agg @ p['go_w'] + p['go_b'])                   # [A,HID]
    # phase competition actor head
    eye = jnp.eye(P, dtype=h2.dtype)
    x = jnp.concatenate([jnp.broadcast_to(h2[:, None, :], (A, P, HID)),
                         jnp.broadcast_to(eye, (A, P, P))], axis=-1)
    pe = _ln(jax.nn.relu(x @ p['pe_w'] + p['pe_b']), p['pe_g'], p['pe_be'])
    hd = PD // PH
    qa = (pe @ p['aq_w'] + p['aq_b']).reshape(A, P, PH, hd)
    ka = (pe @ p['ak_w'] + p['ak_b']).reshape(A, P, PH, hd)
    va = (pe @ p['av_w'] + p['av_b']).reshape(A, P, PH, hd)
    aw = jax.nn.softmax(jnp.einsum('apHd,aqHd->aHpq', qa, ka) / np.sqrt(float(hd)),
                        axis=-1)
    ao = jnp.einsum('aHpq,aqHd->apHd', aw, va).reshape(A, P, PD)
    ao = ao @ p['ao_w'] + p['ao_b']
    actor_logits = (jax.nn.relu(ao @ p['h1_w'] + p['h1_b']) @ p['h2_w']
                    + p['h2_b']).squeeze(-1)                        # [A,P]
    # critic
    g = jnp.broadcast_to(jnp.mean(h2, axis=0, keepdims=True), (A, HID))
    c = jnp.concatenate([h2, g], axis=-1)
    value = (jax.nn.relu(jax.nn.relu(c @ p['c1_w'] + p['c1_b']) @ p['c2_w']
                         + p['c2_b']) @ p['c3_w'] + p['c3_b']).squeeze(-1)
    return actor_logits, value


_pfn = jax.pmap(_per_core, in_axes=(0, 0, 0, None))


def kernel(obs, adj_matrix, phase_axis, params):
    params = {k: jnp.asarray(np.asarray(v)) for k, v in params.items()}
    actor, value = _pfn(jnp.asarray(np.asarray(obs)),
                        jnp.asarray(np.asarray(adj_matrix)),
                        jnp.asarray(np.asarray(phase_axis)),
                        params)
    return np.asarray(actor, dtype=np.float32), np.asarray(value, dtype=np.float32)


# revision 3
# speedup vs baseline: 1.0080x; 1.0080x over previous
"""Trainium kernel for nn_PPOCoLightNetwork.

Strategy: data-parallel over the leading batch dim (B=8) across the 8
NeuronCores — one batch element per core, params replicated. Each core
runs the full per-sample network (embed MLP -> GAT with one-hot neighbor
gather + edge-distance bias -> phase-competition actor head -> critic).
Outputs are gathered back to full shape on host.
"""
import numpy as np
import jax
import jax.numpy as jnp

A, D_IN = 1024, 32
HID = 128
H_DIM, HEADS = 32, 5
P, PD, PH = 8, 64, 4
ROW = COL = 32
N = 5


def _ln(x, g, b, eps=1e-5):
    m = jnp.mean(x, axis=-1, keepdims=True)
    v = jnp.mean(jnp.square(x - m), axis=-1, keepdims=True)
    return (x - m) / jnp.sqrt(v + eps) * g + b


def _per_core(obs, adj_matrix, phase_axis, p):
    # obs [A,D_IN], adj [A,N,A], phase [A]
    h = obs
    for i in range(2):
        h = _ln(jax.nn.relu(h @ p[f'emb_w{i}'] + p[f'emb_b{i}']),
                p[f'emb_g{i}'], p[f'emb_be{i}'])
    # GAT: one-hot neighbor gather — adj rows are one-hot, so a single
    # argmax pass over adj yields indices; gather h rows instead of the
    # dense [A*N,A]@[A,HID] matmul (avoids transposing 21MB on device)
    nidx0 = jnp.argmax(adj_matrix, axis=-1)                        # [A,N]
    nbr = h[nidx0]                                                 # [A,N,HID]
    q = (h @ p['gq_w'] + p['gq_b']).reshape(A, H_DIM, HEADS).transpose(0, 2, 1)
    k = (nbr @ p['gk_w'] + p['gk_b']).reshape(A, N, H_DIM, HEADS).transpose(0, 3, 1, 2)
    v = (nbr @ p['gv_w'] + p['gv_b']).reshape(A, N, H_DIM, HEADS).transpose(0, 3, 1, 2)
    logits = jnp.einsum('ahd,ahnd->ahn', q, k)                     # [A,H,N]
    # edge-distance bias
    idx = jnp.arange(A)
    # coords[m] = (m // 32, m % 32): compute arithmetically, no gather
    nr = (nidx0 // COL).astype(jnp.float32)                         # [A,N]
    ncl = (nidx0 % COL).astype(jnp.float32)
    er = (idx // COL).astype(jnp.float32)[:, None]                  # [A,1]
    ec = (idx % COL).astype(jnp.float32)[:, None]
    dr = nr - er
    dc = ncl - ec
    man = jnp.abs(dr) + jnp.abs(dc)
    dist = man / float(ROW + COL - 2)
    nh = (jnp.abs(dc) >= jnp.abs(dr)).astype(jnp.float32)
    ph = jnp.broadcast_to(phase_axis[:, None], nh.shape)
    same = (nh == ph).astype(jnp.float32)
    conn = (man > 0).astype(jnp.float32)
    eraw = jnp.stack([same, 1.0 - same, dist, conn], axis=-1)       # [A,N,4]
    ebias = jax.nn.relu(eraw @ p['ew1'] + p['eb1']) @ p['ew2'] + p['eb2']
    logits = logits + ebias.transpose(0, 2, 1)
    att = jax.nn.softmax(logits, axis=-1)
    agg = jnp.einsum('ahn,ahnd->ahd', att, v).mean(axis=1)          # [A,H_DIM]
    h2 = jax.nn.relu(agg @ p['go_w'] + p['go_b'])                   # [A,HID]
    # phase competition actor head
    eye = jnp.eye(P, dtype=h2.dtype)
    x = jnp.concatenate([jnp.broadcast_to(h2[:, None, :], (A, P, HID)),
                         jnp.broadcast_to(eye, (A, P, P))], axis=-1)
    pe = _ln(jax.nn.relu(x @ p['pe_w'] + p['pe_b']), p['pe_g'], p['pe_be'])
    hd = PD // PH
    qa = (pe @ p['aq_w'] + p['aq_b']).reshape(A, P, PH, hd)
    ka = (pe @ p['ak_w'] + p['ak_b']).reshape(A, P, PH, hd)
    va = (pe @ p['av_w'] + p['av_b']).reshape(A, P, PH, hd)
    aw = jax.nn.softmax(jnp.einsum('apHd,aqHd->aHpq', qa, ka) / np.sqrt(float(hd)),
                        axis=-1)
    ao = jnp.einsum('aHpq,aqHd->apHd', aw, va).reshape(A, P, PD)
    ao = ao @ p['ao_w'] + p['ao_b']
    actor_logits = (jax.nn.relu(ao @ p['h1_w'] + p['h1_b']) @ p['h2_w']
                    + p['h2_b']).squeeze(-1)                        # [A,P]
    # critic
    g = jnp.broadcast_to(jnp.mean(h2, axis=0, keepdims=True), (A, HID))
    c = jnp.concatenate([h2, g], axis=-1)
    value = (jax.nn.relu(jax.nn.relu(c @ p['c1_w'] + p['c1_b']) @ p['c2_w']
                         + p['c2_b']) @ p['c3_w'] + p['c3_b']).squeeze(-1)
    return actor_logits, value


_pfn = jax.pmap(_per_core, in_axes=(0, 0, 0, None))


def kernel(obs, adj_matrix, phase_axis, params):
    params = {k: jnp.asarray(np.asarray(v)) for k, v in params.items()}
    actor, value = _pfn(jnp.asarray(np.asarray(obs)),
                        jnp.asarray(np.asarray(adj_matrix)),
                        jnp.asarray(np.asarray(phase_axis)),
                        params)
    return np.asarray(actor, dtype=np.float32), np.asarray(value, dtype=np.float32)
